# revision 45
# baseline (speedup 1.0000x reference)
"""GCN (2-layer, PyG GCNConv semantics) on 8 Trainium2 NeuronCores.

Strategy (graph/data parallel, destination-bucketed, gather-based):
  - Nodes sorted by in-degree (desc) and dealt round-robin to the 8
    cores (6250 real + 150 pad each, 50 dest tiles of 128). Sorting
    makes each 128-node dest tile near-uniform in degree, so per-tile
    chunk counts K[t] (shared across cores, SPMD) are tight (~2% pad).
  - Normalization factored per-node: tables hold dinv[v]*h[v]; the
    aggregation is an unweighted sum; layer-1 results fold both dest
    factors into one scale (table2 = dinv^2 * relu(agg), b1==0 path).
  - Aggregation: one dma_gather per dest tile fetches, for each (dest
    partition p, chunk j) slot, the PAIR of table rows [2k, 2k+1]
    where k = v//2 (elem = 512B; pair indices < 25600 fit int16). A
    uint8 parity mask drives one DVE copy_predicated that moves odd
    halves into the consumed position; a strided DVE tensor_reduce
    then sums the chunks per dest - no identity matmuls, no PSUM
    accumulation. Pad slots point at a guaranteed-zero pad pair.
  - Gathers are spread round-robin across 4 SWDGE queues: measured
    ~3 ns/descriptor on TRN2 vs ~8 ns single-queue (per-queue drain
    serialization is the single-queue bottleneck).
  - Both layers' tables are F1=128 wide (W2 is applied after the
    layer-2 aggregation), so one index/mask set serves both layers.
  - Tables are bf16; reduction accumulates fp32 on DVE. Transformed
    tables are AllGathered so every core gathers from a local full
    table; W1/W2 replicated.
"""

import numpy as np
import ml_dtypes

import concourse.bacc as bacc
import concourse.bass as bass
import concourse.mybir as mybir
import concourse.tile as tile
from concourse import bass_utils
from concourse.bass import ts
from concourse.masks import make_identity

N = 50000
F0, F1, F2 = 512, 128, 64
NCORES = 8
NSH = N // NCORES          # 6250 real nodes per core
NP = 6400                  # padded nodes per core (50 tiles of 128)
NT = NP // 128             # 50 dest tiles per core
TBL = NCORES * NP          # 51200 rows in the gathered tables
PAD_PAIR = 3199            # core-0 pad pair (rows 6398/6399, all-zero)
F32 = mybir.dt.float32
BF16 = mybir.dt.bfloat16
I16 = mybir.dt.int16
BF = ml_dtypes.bfloat16

_TRACE = False
_LAST = None               # BassKernelResults of the most recent run


def _wrap16(flat_idx):
    """dma_gather index layout: element i at [i%16, i//16], replicated to
    128 partitions (one copy per GpSimd core)."""
    n = len(flat_idx)
    a = np.zeros((16, n // 16), np.int16)
    i = np.arange(n)
    a[i % 16, i // 16] = flat_idx.astype(np.int16)
    return np.tile(a, (8, 1))


def _host_prep(x, edge_index):
    src = np.asarray(edge_index[0], dtype=np.int64)
    dst = np.asarray(edge_index[1], dtype=np.int64)
    x = np.asarray(x, dtype=np.float32)

    deg = np.bincount(dst, minlength=N) + 1          # with self-loops
    order = np.argsort(-deg, kind="stable")          # rank -> node id
    r = np.arange(N)

    # pass 1: per degree-tile chunk counts (invariant under tile renumber)
    s_nat = r // NCORES                              # within-core natural slot
    pos_nat = (r % NCORES) * NP + s_nat
    pos_of_nat = np.empty(N, np.int64)
    pos_of_nat[order] = pos_nat
    dpos_nat = pos_of_nat[dst]
    cnt_nat = np.bincount(np.concatenate([dpos_nat, pos_nat]), minlength=TBL)
    K_nat = cnt_nat.reshape(NCORES, NT, 128).max(axis=2).max(axis=0)

    # issue order over degree-tiles: few small first (fast pipeline fill),
    # then largest-to-smallest, ending small (short tail). Tiles are then
    # RENUMBERED so issued tile i occupies rows i*128..i*128+127 - local
    # table halves [0:3200) / [3200:6400) complete in issue order, letting
    # the AllGathers split in two and overlap with compute.
    tiles = [t for t in range(NT) if K_nat[t] > 0]
    by_k = sorted(tiles, key=lambda t: K_nat[t])
    ORDER_D = by_k[:5] + by_k[10:][::-1] + by_k[5:10]
    ORDER_FULL = ORDER_D + [t for t in range(NT) if K_nat[t] == 0]
    inv = np.empty(NT, np.int64)
    inv[np.asarray(ORDER_FULL)] = np.arange(NT)

    # remapped positions: node at natural slot s -> issued tile inv[s//128]
    lp_of_s = inv[s_nat // 128] * 128 + (s_nat % 128)
    pos_of = np.empty(N, np.int64)                   # node -> c*NP + lp
    pos_of[order] = (r % NCORES) * NP + lp_of_s

    all_src = np.concatenate([src, np.arange(N, dtype=np.int64)])
    all_dst = np.concatenate([dst, np.arange(N, dtype=np.int64)])
    dpos = pos_of[all_dst]
    spos = pos_of[all_src]

    # table-row encoding for split AllGathers: local rows are AllGathered
    # in 4 parts (tile ranges [0:12], [12:25], [25:37], [37:50]); part p of
    # core c lands at POFF[p] + c*PROWS[p] + (lp - PSTART[p])
    PSTART = np.array([0, 3200, NP])
    PROWS = np.diff(PSTART)
    POFF = np.concatenate([[0], np.cumsum(NCORES * PROWS)])

    def enc(pos):
        c = pos // NP
        lp = pos % NP
        h = np.searchsorted(PSTART, lp, side="right") - 1
        return POFF[h] + c * PROWS[h] + (lp - PSTART[h])

    senc = enc(spos)

    o = np.argsort(dpos, kind="stable")
    sd = dpos[o]
    se = senc[o]
    st = np.searchsorted(sd, np.arange(TBL))
    j = np.arange(len(sd)) - st[sd]
    cnt = np.bincount(dpos, minlength=TBL)
    Kct = cnt.reshape(NCORES, NT, 128).max(axis=2)
    K0 = Kct.max(axis=0).astype(np.int64)
    # pad pair: remapped rows of core-0 natural slots 6250/6251 (both zero,
    # adjacent within the same renumbered tile, even-aligned)
    pad_lp = int(inv[6250 // 128] * 128 + 6250 % 128)
    pad_enc = int(enc(np.asarray([pad_lp]))[0])
    assert pad_enc % 2 == 0
    PADP = pad_enc // 2

    offs0 = np.concatenate([[0], np.cumsum(K0)]).astype(np.int64)
    idx0 = np.full((NCORES, 128, int(offs0[-1])), PADP, dtype=np.int64)
    msk0 = np.full((NCORES, 128, int(offs0[-1])),
                   pad_enc % 2, dtype=np.uint8)
    c_of = sd // NP
    lp = sd % NP
    idx0[c_of, lp % 128, offs0[lp // 128] + j] = se // 2
    msk0[c_of, lp % 128, offs0[lp // 128] + j] = se % 2

    ORDER = [t for t in range(NT) if K0[t] > 0]

    def wrap_core(idx_c):
        blocks = []
        for t in ORDER:
            blk = idx_c[:, offs0[t]:offs0[t + 1]]     # [128, K[t]]
            flat = blk.T.reshape(-1)                  # i = j*128 + p
            blocks.append(_wrap16(flat))
        return np.ascontiguousarray(np.concatenate(blocks, axis=1))

    def mask_core(msk_c):
        cols = [msk_c[:, offs0[t]:offs0[t + 1]] for t in ORDER]
        return np.ascontiguousarray(np.concatenate(cols, axis=1))

    c_arr = pos_of // NP
    lp_arr = pos_of % NP
    in_maps = []
    for c in range(NCORES):
        sel = np.nonzero(c_arr == c)[0]
        xp = np.zeros((NP, F0), dtype=np.float32)
        xp[lp_arr[sel]] = x[sel]
        xt4 = np.ascontiguousarray(xp.T.reshape(4, 128, NP)).astype(BF)
        degp = np.ones(NP, dtype=np.float32)
        degp[lp_arr[sel]] = deg[sel].astype(np.float32)
        degT = np.ascontiguousarray(degp.reshape(NT, 128).T)  # [128, NT]
        m = {
            "xt4": xt4,
            "degT": degT,
            "idx0": wrap_core(idx0[c]),
            "mskp": mask_core(msk0[c]),
        }
        in_maps.append(m)

    return in_maps, pos_of, K0, ORDER


def _build(K0, ORDER, w0n, w1n, b1_zero, b2_zero):
    Relu = mybir.ActivationFunctionType.Relu
    Copy = mybir.ActivationFunctionType.Copy
    Sqrt = mybir.ActivationFunctionType.Sqrt

    nc = bacc.Bacc("TRN2", target_bir_lowering=False, num_devices=NCORES,
                   num_swdge_queues=4)

    xt4_d = nc.dram_tensor("xt4", [4, 128, NP], BF16, kind="ExternalInput")
    w1r_d = nc.dram_tensor("w1r", [4, 128, F1], BF16, kind="ExternalInput")
    w2_d = nc.dram_tensor("w2", [F1, F2], BF16, kind="ExternalInput")
    degT_d = nc.dram_tensor("degT", [128, NT], F32, kind="ExternalInput")
    i0_d = nc.dram_tensor("idx0", [128, w0n], I16, kind="ExternalInput")
    mk_d = nc.dram_tensor("mskp", [128, w1n], mybir.dt.uint8,
                          kind="ExternalInput")
    b1r_d = b2r_d = None
    if not b1_zero:
        b1r_d = nc.dram_tensor("b1r", [128, F1], F32, kind="ExternalInput")
    if not b2_zero:
        b2r_d = nc.dram_tensor("b2r", [128, F2], F32, kind="ExternalInput")
    out_d = nc.dram_tensor("out", [NP, F2], F32, kind="ExternalOutput")

    # AllGather split: local tile ranges -> contiguous shared-table parts
    PSTART = [0, 3200, NP]
    NPARTS = len(PSTART) - 1
    POFF = [0]
    for i in range(NPARTS):
        POFF.append(POFF[-1] + NCORES * (PSTART[i + 1] - PSTART[i]))
    h1_loc = nc.dram_tensor("h1_loc", [NP, F1], BF16, kind="Internal")
    h1_full = nc.dram_tensor("h1_full", [TBL, F1], BF16, kind="Internal",
                             addr_space="Shared")
    h2_loc = nc.dram_tensor("h2_loc", [NP, F1], BF16, kind="Internal")
    h2_full = nc.dram_tensor("h2_full", [TBL, F1], BF16, kind="Internal",
                             addr_space="Shared")

    def ag_part(loc, full, p):
        nc.gpsimd.collective_compute(
            "AllGather", mybir.AluOpType.bypass, replica_groups=rg,
            ins=[loc[PSTART[p]:PSTART[p + 1]]],
            outs=[full[POFF[p]:POFF[p + 1]]])

    rg = [list(range(NCORES))]
    K = np.asarray(K0).astype(np.int64)
    KCAP = 32                 # sub-gather chunk cap (smaller msg tiles)

    with tile.TileContext(nc, num_cores=NCORES) as tc:
        with (
            tc.tile_pool(name="const", bufs=1) as cpool,
            tc.tile_pool(name="stream", bufs=3) as spool,
            tc.tile_pool(name="msg", bufs=8) as mpool,
            tc.tile_pool(name="red", bufs=4) as rpool,
            tc.tile_pool(name="psB", bufs=2, space="PSUM") as psB,
            tc.tile_pool(name="psA", bufs=2, space="PSUM") as psA,
            tc.tile_pool(name="psT", bufs=2, space="PSUM") as psT,
            tc.tile_pool(name="psW", bufs=2, space="PSUM") as psW,
        ):
            # ---- constants -------------------------------------------------
            w1sb = cpool.tile([128, 4, F1], BF16)
            nc.sync.dma_start(out=w1sb[:], in_=w1r_d[:].rearrange("k p f -> p k f"))
            w2sb = cpool.tile([128, F2], BF16)
            nc.sync.dma_start(out=w2sb[:], in_=w2_d[:])
            degsb = cpool.tile([128, NT], F32)
            nc.sync.dma_start(out=degsb[:], in_=degT_d[:])
            i0sb = cpool.tile([128, w0n], I16)
            nc.sync.dma_start(out=i0sb[:], in_=i0_d[:])
            mksb = cpool.tile([128, w1n], mybir.dt.uint8)
            nc.sync.dma_start(out=mksb[:], in_=mk_d[:])
            identf = cpool.tile([128, 128], F32)
            make_identity(nc, identf[:])
            identb = cpool.tile([128, 128], BF16)
            nc.scalar.copy(identb[:], identf[:])
            b1sb = b2sb = None
            if not b1_zero:
                b1sb = cpool.tile([128, F1], F32)
                nc.sync.dma_start(out=b1sb[:], in_=b1r_d[:])
            if not b2_zero:
                b2sb = cpool.tile([128, F2], F32)
                nc.sync.dma_start(out=b2sb[:], in_=b2r_d[:])

            rec = cpool.tile([128, NT], F32)
            nc.vector.reciprocal(rec[:], degsb[:])
            dinv = cpool.tile([128, NT], F32)
            nc.scalar.activation(dinv[:], rec[:], Sqrt)
            dinv2 = cpool.tile([128, NT], F32)
            nc.vector.tensor_tensor(out=dinv2[:], in0=dinv[:], in1=dinv[:],
                                    op=mybir.AluOpType.mult)

            # ---- phase B: h1 = dinv_src * (x @ W1), node-major -------------
            # AllGather fires per table half as soon as its tiles are done.
            BBLK = 5                    # tiles per x stream block
            for t0 in range(0, NT, BBLK):
                nb = min(BBLK, NT - t0)
                xt = spool.tile([128, 4, BBLK * 128], BF16, tag="xt")
                nc.sync.dma_start(
                    out=xt[:, :, :nb * 128],
                    in_=xt4_d[:, :, ts(t0 // BBLK, BBLK * 128)].rearrange(
                        "k p n -> p k n"))
                for tt in range(nb):
                    t = t0 + tt
                    ph = psB.tile([128, F1], F32, tag="ph")
                    for k in range(4):
                        nc.tensor.matmul(ph[:], lhsT=xt[:, k, ts(tt, 128)],
                                         rhs=w1sb[:, k, :],
                                         start=(k == 0), stop=(k == 3))
                    h1t = spool.tile([128, F1], BF16, tag="h1t")
                    nc.scalar.activation(h1t[:], ph[:], Copy,
                                         scale=dinv[:, t:t + 1])
                    nc.sync.dma_start(out=h1_loc[ts(t, 128), :], in_=h1t[:])
                    if 128 * (t + 1) in PSTART[1:-1]:
                        ag_part(h1_loc, h1_full,
                                PSTART.index(128 * (t + 1)) - 1)
            ag_part(h1_loc, h1_full, NPARTS - 1)

            # ---- pair-row gather AP: idx k fetches table rows [2k, 2k+1] ---
            def pair_ap(table):
                a = table[:]
                return bass.AP(a.tensor, a.offset,
                               [[2 * F1, TBL // 2], [1, 2 * F1]])

            qrr = [0]

            # per dest tile: gather pairs (split at KCAP chunks per gather),
            # parity-select (DVE), then accumulate: tiles alternate between
            # a DVE strided tensor_reduce and PE identity-matmul PSUM
            # accumulation to split the consume load across engines.
            def aggregate(table, consume, mid_hook=None):
                o0 = [0]
                om = [0]
                ap0 = pair_ap(table)

                def gather_sub(kk):
                    n = 128 * kk
                    msg = mpool.tile([128, KCAP, 2 * F1], BF16, tag="msg")
                    nc.gpsimd.dma_gather(
                        out_ap=msg[:, :kk, :], in_ap=ap0,
                        idxs_ap=i0sb[:, o0[0]:o0[0] + n // 16],
                        num_idxs=n, num_idxs_reg=n,
                        elem_size=2 * F1, elem_step=2 * F1,
                        single_packet=False,
                        queue_num=qrr[0] % 4)
                    qrr[0] += 1
                    o0[0] += n // 16
                    mka = mksb[:, om[0]:om[0] + kk]
                    mask_b = bass.AP(mka.tensor, mka.offset,
                                     [mka.ap[0], mka.ap[1], [0, F1]])
                    nc.vector.copy_predicated(
                        msg[:, :kk, 0:F1], mask_b,
                        msg[:, :kk, F1:2 * F1])
                    om[0] += kk
                    return msg

                for ti, t in enumerate(ORDER):
                    kt = int(K[t])
                    nsub = (kt + KCAP - 1) // KCAP
                    ksub = (kt + nsub - 1) // nsub
                    sizes = []
                    lo = 0
                    while lo < kt:
                        sizes.append(min(ksub, kt - lo))
                        lo += ksub
                    use_pe = (ti % 5) < 3         # 3:2 tile split PE / DVE
                    if use_pe:
                        po = psA.tile([128, F1], F32, tag="po")
                        c0 = 0
                        for kk in sizes:
                            msg = gather_sub(kk)
                            for j in range(kk):
                                nc.tensor.matmul(po[:], lhsT=identb[:],
                                                 rhs=msg[:, j, 0:F1],
                                                 start=(c0 == 0),
                                                 stop=(c0 == kt - 1))
                                c0 += 1
                        consume(t, po)
                    else:
                        parts = []
                        for kk in sizes:
                            msg = gather_sub(kk)
                            ra = rpool.tile([128, F1], F32, tag="ra")
                            nc.vector.tensor_reduce(
                                out=ra[:],
                                in_=msg[:, 0:kk, 0:F1].rearrange(
                                    "p k f -> p f k"),
                                axis=mybir.AxisListType.X,
                                op=mybir.AluOpType.add)
                            parts.append(ra)
                        while len(parts) > 1:
                            rs = rpool.tile([128, F1], F32, tag="rs")
                            nc.vector.tensor_tensor(
                                out=rs[:], in0=parts[0][:], in1=parts[1][:],
                                op=mybir.AluOpType.add)
                            parts = [rs] + parts[2:]
                        consume(t, parts[0])
                    if mid_hook is not None and 128 * (t + 1) in PSTART[1:-1]:
                        mid_hook(PSTART.index(128 * (t + 1)) - 1)

            # ---- L1 consume: table2 = dinv^2 * relu(agg)  (b1==0 path) -----
            def consume1(t, ra):
                h2t = spool.tile([128, F1], BF16, tag="h2t")
                if b1_zero:
                    nc.scalar.activation(h2t[:], ra[:], Relu,
                                         scale=dinv2[:, t:t + 1])
                else:
                    s = spool.tile([128, F1], F32, tag="s1")
                    nc.scalar.activation(s[:], ra[:], Copy,
                                         scale=dinv[:, t:t + 1])
                    s2 = spool.tile([128, F1], F32, tag="s2")
                    nc.vector.tensor_tensor(out=s2[:], in0=s[:], in1=b1sb[:],
                                            op=mybir.AluOpType.add)
                    r = spool.tile([128, F1], F32, tag="r1")
                    nc.scalar.activation(r[:], s2[:], Relu)
                    nc.scalar.activation(h2t[:], r[:], Copy,
                                         scale=dinv[:, t:t + 1])
                nc.sync.dma_start(out=h2_loc[ts(t, 128), :], in_=h2t[:])

            # zero-degree (all-pad) tiles still need zero table rows
            zt = None
            for t in range(NT):
                if int(K[t]) == 0:
                    if zt is None:
                        zt = cpool.tile([128, F1], BF16)
                        nc.vector.memset(zt[:], 0.0)
                    nc.sync.dma_start(out=h2_loc[ts(t, 128), :], in_=zt[:])

            aggregate(h1_full, consume1,
                      mid_hook=lambda p: ag_part(h2_loc, h2_full, p))

            # ---- AllGather table2 (last part) ------------------------------
            ag_part(h2_loc, h2_full, NPARTS - 1)

            # ---- L2 consume: out = dinv * (agg @ W2) + b2 ------------------
            def consume2(t, ra):
                cb = spool.tile([128, F1], BF16, tag="cb")
                nc.scalar.copy(cb[:], ra[:])
                pT = psT.tile([128, 128], BF16, tag="pT")
                nc.tensor.transpose(pT[:], cb[:], identb[:])
                o1T = spool.tile([128, F1], BF16, tag="o1T")
                nc.scalar.copy(o1T[:], pT[:])
                po = psW.tile([128, F2], F32, tag="po")
                nc.tensor.matmul(po[:], lhsT=o1T[:], rhs=w2sb[:],
                                 start=True, stop=True)
                o2t = spool.tile([128, F2], F32, tag="o2t")
                nc.scalar.activation(o2t[:], po[:], Copy,
                                     scale=dinv[:, t:t + 1])
                if not b2_zero:
                    nc.vector.tensor_tensor(out=o2t[:], in0=o2t[:],
                                            in1=b2sb[:],
                                            op=mybir.AluOpType.add)
                nc.sync.dma_start(out=out_d[ts(t, 128), :], in_=o2t[:])

            zo = None
            for t in range(NT):
                if int(K[t]) == 0:
                    if zo is None:
                        zo = cpool.tile([128, F2], F32)
                        nc.vector.memset(zo[:], 0.0)
                        if not b2_zero:
                            nc.vector.tensor_tensor(
                                out=zo[:], in0=zo[:], in1=b2sb[:],
                                op=mybir.AluOpType.add)
                    nc.sync.dma_start(out=out_d[ts(t, 128), :], in_=zo[:])

            aggregate(h2_full, consume2)

    nc.compile()
    return nc


def kernel(x, edge_index, W1, b1, W2, b2):
    global _LAST
    b1 = np.asarray(b1, np.float32)
    b2 = np.asarray(b2, np.float32)
    in_maps, pos_of, K0, ORDER = _host_prep(x, edge_index)

    b1_zero = bool(np.all(b1 == 0))
    b2_zero = bool(np.all(b2 == 0))
    for m in in_maps:
        m["w1r"] = np.ascontiguousarray(
            np.asarray(W1, np.float32).reshape(4, 128, F1)).astype(BF)
        m["w2"] = np.ascontiguousarray(np.asarray(W2, np.float32)).astype(BF)
        if not b1_zero:
            m["b1r"] = np.ascontiguousarray(np.tile(b1[None, :], (128, 1)))
        if not b2_zero:
            m["b2r"] = np.ascontiguousarray(np.tile(b2[None, :], (128, 1)))

    w0n = in_maps[0]["idx0"].shape[1]
    w1n = in_maps[0]["mskp"].shape[1]
    nc = _build(K0, ORDER, w0n, w1n, b1_zero, b2_zero)

    res = bass_utils.run_bass_kernel_spmd(
        nc, in_maps, core_ids=list(range(NCORES)), trace=_TRACE)
    _LAST = res

    out = np.empty((N, F2), dtype=np.float32)
    c_arr = pos_of // NP
    lp_arr = pos_of % NP
    for c in range(NCORES):
        sel = np.nonzero(c_arr == c)[0]
        out[sel] = res.results[c]["out"][lp_arr[sel]]
    return out
